# revision 10
# baseline (speedup 1.0000x reference)
# Trainium2 Bass kernel for nn_LogitsNew (dense_mlp).
#
#   u = gelu(x @ W_proj + b_proj)                       [B, D]
#   logits = (u @ W_u)[:, None, :] + ee @ W_e           [B, N, C]
#
# Sharding: data-parallel over batch B across 8 cores (4 batches/core).
#
# fp16 end-to-end (tolerance 2e-2; fp16 keeps rel err ~4e-4), host-side
# layout transforms (k-chunk layouts, no PE transposes for the main path),
# fp16 stores (upcast on host). ~10MB HBM traffic per core, which at the
# ~230GB/s effective per-core rate (8 cores concurrently hammering HBM)
# is ~44us -- matched against ~40us of PE work.
#
# Per core (PE order): mt0..mt2 | z,u | mt3 | y,ybc | epilogue(mt0-3) |
# mt4..7 (fused DVE add-drains + immediate stores).
#   - main m-tile: accumulate eeT.T @ W_e into two PSUM banks (8 k each).
#   - y broadcast: 8 PE "selector" matmuls (lhsT = e_b x ones_128, rhs =
#     y[4, 512] fp16) materialize ybc[128, b, c] in f32; every drain of a
#     late m-tile is then a single DVE tensor_add(psum, ybc) -> fp16.
#   - early m-tiles (psum drained before y exists) get the same add in a
#     hoisted epilogue that runs on the DVE while the PE is on mt4..7.
# DMA rings (HWDGE only): sync: W_e x4, W_u x2 + even stores; scalar:
# ee0ab, ee1ab, x, b, sel, wp x2, ee2, ee3 + odd stores.

import sys

if "/opt/trn_rl_repo" not in sys.path:
    sys.path.insert(0, "/opt/trn_rl_repo")

import numpy as np

import concourse.bass as bass
import concourse.mybir as mybir
import concourse.tile as tile
from concourse import bacc
from concourse.bass_utils import run_bass_kernel_spmd
from concourse.masks import make_identity

P = 128
B, N, D, C = 32, 256, 1024, 1024
NCORES = 8
BPC = B // NCORES          # batches per core
KT = D // P                # 8 k-tiles over the contraction dim
FD = 512                   # matmul moving free dim (one PSUM bank of fp32)
NT = N // P                # 2 n-tiles per batch
MT = BPC * NT              # 8 m-tiles per core
NEARLY = 4                 # m-tiles drained before y exists (epilogue add)

F32 = mybir.dt.float32
F16 = mybir.dt.float16
GELU = mybir.ActivationFunctionType.Gelu

_CACHE = {}


def _build():
    if "nc" in _CACHE:
        return _CACHE["nc"]

    nc = bacc.Bacc("TRN2", target_bir_lowering=False, debug=False, num_devices=NCORES)

    # host-transformed inputs (fp16, k-chunk layouts)
    eet = nc.dram_tensor("eet", [BPC, P, KT, N], F16, kind="ExternalInput").ap()
    we = nc.dram_tensor("we", [P, KT, C], F16, kind="ExternalInput").ap()
    wu = nc.dram_tensor("wu", [P, KT, C], F16, kind="ExternalInput").ap()
    wp = nc.dram_tensor("wp", [P, KT, C], F16, kind="ExternalInput").ap()
    xt = nc.dram_tensor("xt", [P, KT, BPC], F16, kind="ExternalInput").ap()
    bp = nc.dram_tensor("bp", [1, D], F16, kind="ExternalInput").ap()
    seld = nc.dram_tensor("sel", [BPC, BPC * P], F16, kind="ExternalInput").ap()
    out = nc.dram_tensor("logits", [BPC, N, C], F16, kind="ExternalOutput").ap()

    with tile.TileContext(nc) as tc:
        with (
            tc.tile_pool(name="const", bufs=1) as cpool,
            tc.tile_pool(name="outs", bufs=1) as outpool,
            tc.tile_pool(name="ost", bufs=4) as ostpool,
            tc.tile_pool(name="tp_ps", bufs=2, space="PSUM") as tp_ps,
            tc.tile_pool(name="mm_ps", bufs=6, space="PSUM") as mm_ps,
        ):
            # ---- W_e then W_u on the sync ring, fine-grained tiles ----
            wesb = []
            for j in range(4):
                t = cpool.tile([P, 2, C], F16, name=f"we_{j}")
                nc.sync.dma_start(t, we[:, 2 * j : 2 * j + 2])
                wesb.append(t)
            wusb = []
            for j in range(2):
                t = cpool.tile([P, 4, C], F16, name=f"wu_{j}")
                nc.sync.dma_start(t, wu[:, 4 * j : 4 * j + 4])
                wusb.append(t)

            # ---- ee batches / x / b / sel / W_proj on the scalar ring, in
            # consumption order ----
            eesb = {}  # (b, half) -> tile [P, 4, N]
            def ee_load(b, half):
                t = cpool.tile([P, 4, N], F16, name=f"ee_{b}_{half}")
                nc.scalar.dma_start(t, eet[b, :, 4 * half : 4 * half + 4])
                eesb[(b, half)] = t

            ee_load(0, 0)
            ee_load(0, 1)
            ee_load(1, 0)
            xsb = cpool.tile([P, KT, BPC], F16)
            nc.scalar.dma_start(xsb, xt)
            bsb = cpool.tile([1, D], F16)
            nc.scalar.dma_start(bsb, bp)
            sel = cpool.tile([BPC, BPC * P], F16)
            nc.scalar.dma_start(sel, seld)
            wpsb = []
            for j in range(2):
                t = cpool.tile([P, 4, C], F16, name=f"wp_{j}")
                if j == 0:
                    nc.scalar.dma_start(t, wp[:, :4])
                    wpsb.append(t)
            ee_load(1, 1)
            t = cpool.tile([P, 4, C], F16, name="wp_1")
            nc.scalar.dma_start(t, wp[:, 4:])
            wpsb.append(t)
            ee_load(2, 0)
            ee_load(2, 1)
            ee_load(3, 0)
            ee_load(3, 1)

            # ---- constants ----
            ident_f = cpool.tile([P, P], F32)
            make_identity(nc, ident_f)
            ident = cpool.tile([P, P], F16)
            nc.scalar.copy(ident, ident_f)
            ones_f = cpool.tile([1, P], F32)
            nc.gpsimd.memset(ones_f, 1.0)
            ones = cpool.tile([1, P], F16)
            nc.scalar.copy(ones, ones_f)

            usb = cpool.tile([BPC, C], F16)
            uT = cpool.tile([P, KT, BPC], F16)
            ysb = cpool.tile([BPC, C], F16)
            ybc = cpool.tile([P, BPC, C], F32)

            o32 = []

            def utter_zu():
                # z = x @ W_proj + b; u = gelu(z)
                for h in range(2):
                    cs = slice(h * FD, (h + 1) * FD)
                    zp = mm_ps.tile([P, FD], F32, tag="mm", name=f"z_{h}")
                    for k in range(KT):
                        nc.tensor.matmul(
                            zp[:BPC], xsb[:, k, :], wpsb[k // 4][:, k % 4, cs],
                            start=(k == 0), stop=False,
                        )
                    nc.tensor.matmul(
                        zp[:BPC], ones[:1, :BPC], bsb[:1, cs],
                        start=False, stop=True,
                    )
                    nc.scalar.activation(usb[:, cs], zp[:BPC], GELU)
                # transpose u (tiny: 8x [4,128] PE transposes)
                for k in range(KT):
                    tp = tp_ps.tile([P, P], F16, tag="tp")
                    nc.tensor.transpose(
                        tp[:, :BPC],
                        usb[:BPC, k * P : (k + 1) * P],
                        ident[:BPC, :BPC],
                    )
                    nc.scalar.copy(uT[:, k, :], tp[:, :BPC])

            def utter_y():
                # y = u @ W_u -> fp16 in partitions 0..3
                for h in range(2):
                    cs = slice(h * FD, (h + 1) * FD)
                    yp = mm_ps.tile([P, FD], F32, tag="mm", name=f"y_{h}")
                    for k in range(KT):
                        nc.tensor.matmul(
                            yp[:BPC], uT[:, k, :], wusb[k // 4][:, k % 4, cs],
                            start=(k == 0), stop=(k == KT - 1),
                        )
                    nc.vector.tensor_copy(ysb[:, cs], yp[:BPC])
                # ybc[:, b, :] = y[b] broadcast to 128 partitions (PE matmul
                # against the e_b x ones_128 selector)
                for b2 in range(BPC):
                    for ch in range(2):
                        cs = slice(ch * FD, (ch + 1) * FD)
                        bp_ = mm_ps.tile([P, FD], F32, tag="mm", name=f"yb{b2}{ch}")
                        nc.tensor.matmul(
                            bp_, sel[:, b2 * P : (b2 + 1) * P], ysb[:BPC, cs],
                            start=True, stop=True,
                        )
                        if ch == 0:
                            nc.scalar.copy(ybc[:, b2, cs], bp_)
                        else:
                            nc.vector.tensor_copy(ybc[:, b2, cs], bp_)

            def store(mt, o):
                b, nh = divmod(mt, NT)
                ns = slice(nh * P, (nh + 1) * P)
                eng = nc.sync if mt % 2 == 0 else nc.scalar
                eng.dma_start(out[b, ns, :], o.rearrange("p a f -> p (a f)"))

            for mt in range(MT):
                if mt == 3:
                    utter_zu()
                if mt == 4:
                    utter_y()
                    # hoisted epilogue: add y to the early tiles on the DVE
                    # and store them, while the PE works on mt4..7
                    for emt in range(NEARLY):
                        eb = emt // NT
                        o = ostpool.tile([P, 2, FD], F16, tag="ost", name=f"oste{emt}")
                        nc.vector.tensor_add(
                            o[:, 0, :], o32[emt][:, 0, :], ybc[:, eb, 0:FD]
                        )
                        nc.vector.tensor_add(
                            o[:, 1, :], o32[emt][:, 1, :], ybc[:, eb, FD:C]
                        )
                        store(emt, o)

                b, nh = divmod(mt, NT)
                ns = slice(nh * P, (nh + 1) * P)
                mps = [
                    mm_ps.tile([P, FD], F32, tag="mm", name=f"mm_{mt}_{ch}")
                    for ch in range(2)
                ]
                for ch in range(2):
                    cs = slice(ch * FD, (ch + 1) * FD)
                    for k in range(KT):
                        nc.tensor.matmul(
                            mps[ch],
                            eesb[(b, k // 4)][:, k % 4, ns],
                            wesb[k // 2][:, k % 2, cs],
                            start=(k == 0),
                            stop=(k == KT - 1),
                        )
                if mt < NEARLY:
                    # y not ready: drain to f32, add y in the hoisted epilogue
                    o = outpool.tile([P, 2, FD], F32, tag=f"o{mt}")
                    nc.scalar.copy(o[:, 0, :], mps[0])
                    nc.vector.tensor_copy(o[:, 1, :], mps[1])
                    o32.append(o)
                else:
                    # fused drain: out = psum + ybc (f32+f32 -> fp16), store
                    o = ostpool.tile([P, 2, FD], F16, tag="ost", name=f"ost{mt}")
                    nc.vector.tensor_add(o[:, 0, :], mps[0], ybc[:, b, 0:FD])
                    nc.vector.tensor_add(o[:, 1, :], mps[1], ybc[:, b, FD:C])
                    store(mt, o)

    nc.compile()
    _CACHE["nc"] = nc
    return nc


def _prep(inputs):
    """Host-side cast to fp16 + k-chunk layout transforms."""
    x = np.asarray(inputs["encoded_utterance"], np.float32)
    ee = np.asarray(inputs["element_embeddings"], np.float32)
    w = np.asarray(inputs["weight_matrix"], np.float32)
    wp = np.asarray(inputs["W_proj"], np.float32)
    bp = np.asarray(inputs["b_proj"], np.float32).reshape(1, D)

    # eet[b, p, k, n] = ee[b, n, k*128+p]
    eet = np.ascontiguousarray(
        ee.reshape(B, N, KT, P).transpose(0, 3, 2, 1)
    ).astype(np.float16)

    # we/wu/wp[p, k, c] = W[k*128+p, c]
    def kchunk(m):
        return np.ascontiguousarray(
            m.reshape(KT, P, C).transpose(1, 0, 2)
        ).astype(np.float16)

    we_h = kchunk(w[D:])
    wu_h = kchunk(w[:D])
    wp_h = kchunk(wp)
    bp_h = bp.astype(np.float16)
    # xt[p, k, b] = x[b, k*128+p], per-core slice of b
    xt_full = np.ascontiguousarray(
        x.reshape(B, KT, P).transpose(2, 1, 0)
    ).astype(np.float16)
    sel_h = np.kron(np.eye(BPC), np.ones((1, P))).astype(np.float16)
    return eet, we_h, wu_h, wp_h, bp_h, xt_full, sel_h


def run(inputs, trace=False, **kwargs):
    nc = _build()
    eet, we_h, wu_h, wp_h, bp_h, xt_full, sel_h = _prep(inputs)

    in_maps = []
    for i in range(NCORES):
        bs = slice(i * BPC, (i + 1) * BPC)
        in_maps.append(
            {
                "eet": np.ascontiguousarray(eet[bs]),
                "we": we_h,
                "wu": wu_h,
                "wp": wp_h,
                "xt": np.ascontiguousarray(xt_full[:, :, bs]),
                "bp": bp_h,
                "sel": sel_h,
            }
        )

    res = run_bass_kernel_spmd(
        nc, in_maps, core_ids=list(range(NCORES)), trace=trace, **kwargs
    )
    full = np.concatenate([r["logits"] for r in res.results], axis=0)
    return full.astype(np.float32), res


def kernel(**inputs) -> np.ndarray:
    return run(inputs, trace=False)[0]


# revision 11
# speedup vs baseline: 1.0436x; 1.0436x over previous
# Trainium2 Bass kernel for nn_LogitsNew (dense_mlp).
#
#   u = gelu(x @ W_proj + b_proj)                       [B, D]
#   logits = (u @ W_u)[:, None, :] + ee @ W_e           [B, N, C]
#
# Sharding: data-parallel over batch B across 8 cores (4 batches/core).
#
# fp16 end-to-end (tolerance 2e-2; fp16 keeps rel err ~4e-4), host-side
# layout transforms (k-chunk layouts, no PE transposes for the main path),
# fp16 stores (upcast on host). ~10MB HBM traffic per core; the two HWDGE
# rings process transfers serially with ~0.6us fixed cost each, so inputs
# ride in FEW, LARGE transfers (0.5-2MB), ordered by consumption time.
#
# Per core (PE order): mt0..mt2 | z,uT | mt3 | y,ybc | epilogue(mt0-3) |
# mt4..7 (sel-fused, drained by scalar+vector copies, stored immediately).
#   - main m-tile: accumulate eeT.T @ W_e into two PSUM banks (8 k each).
#   - z is transposed on the PE into ONE psum tile (8x [4,128] transposes
#     at disjoint column offsets), one Gelu produces uT [128, 8*4] directly.
#   - late m-tiles: a selector matmul (lhsT = e_b x ones_128, rhs =
#     y[4, 512] fp16) is appended to the PSUM group, so PSUM holds the
#     final logits.  Early m-tiles (drained to f32 before y exists) get
#     ybc via 4 PE broadcast-matmuls + DVE adds in a hoisted epilogue that
#     runs while the PE is on mt4..7.

import sys

if "/opt/trn_rl_repo" not in sys.path:
    sys.path.insert(0, "/opt/trn_rl_repo")

import numpy as np

import concourse.bass as bass
import concourse.mybir as mybir
import concourse.tile as tile
from concourse import bacc
from concourse.bass_utils import run_bass_kernel_spmd
from concourse.masks import make_identity

P = 128
B, N, D, C = 32, 256, 1024, 1024
NCORES = 8
BPC = B // NCORES          # batches per core
KT = D // P                # 8 k-tiles over the contraction dim
FD = 512                   # matmul moving free dim (one PSUM bank of fp32)
NT = N // P                # 2 n-tiles per batch
MT = BPC * NT              # 8 m-tiles per core
NEARLY = 4                 # m-tiles drained before y exists (epilogue add)

F32 = mybir.dt.float32
F16 = mybir.dt.float16
GELU = mybir.ActivationFunctionType.Gelu

_CACHE = {}


def _build():
    if "nc" in _CACHE:
        return _CACHE["nc"]

    nc = bacc.Bacc("TRN2", target_bir_lowering=False, debug=False, num_devices=NCORES)

    # host-transformed inputs (fp16, k-chunk layouts, eet partition-major)
    eet = nc.dram_tensor("eet", [P, BPC, KT, N], F16, kind="ExternalInput").ap()
    we = nc.dram_tensor("we", [P, KT, C], F16, kind="ExternalInput").ap()
    wu = nc.dram_tensor("wu", [P, KT, C], F16, kind="ExternalInput").ap()
    wp = nc.dram_tensor("wp", [P, KT, C], F16, kind="ExternalInput").ap()
    xt = nc.dram_tensor("xt", [P, KT, BPC], F16, kind="ExternalInput").ap()
    bp = nc.dram_tensor("bp", [1, D], F16, kind="ExternalInput").ap()
    seld = nc.dram_tensor("sel", [BPC, BPC * P], F16, kind="ExternalInput").ap()
    out = nc.dram_tensor("logits", [BPC, N, C], F16, kind="ExternalOutput").ap()

    with tile.TileContext(nc) as tc:
        with (
            tc.tile_pool(name="const", bufs=1) as cpool,
            tc.tile_pool(name="outs", bufs=1) as outpool,
            tc.tile_pool(name="ost", bufs=4) as ostpool,
            tc.tile_pool(name="tp_ps", bufs=1, space="PSUM") as tp_ps,
            tc.tile_pool(name="mm_ps", bufs=7, space="PSUM") as mm_ps,
        ):
            # ---- sync ring: W_e (3 pieces, earliest consumer) then W_u ----
            wesb = []   # k01 [P,2,C], k23 [P,2,C], k4-7 [P,4,C]
            for j, (k0, nk) in enumerate([(0, 2), (2, 2), (4, 4)]):
                t = cpool.tile([P, nk, C], F16, name=f"we_{j}")
                nc.sync.dma_start(t, we[:, k0 : k0 + nk])
                wesb.append(t)

            def we_at(k):
                if k < 2:
                    return wesb[0][:, k]
                if k < 4:
                    return wesb[1][:, k - 2]
                return wesb[2][:, k - 4]

            wusb = cpool.tile([P, KT, C], F16)
            nc.sync.dma_start(wusb, wu)

            # ---- scalar ring: ee0, ee1, x/b/sel, W_proj, ee2+ee3 ----
            ee01 = []
            for b in range(2):
                t = cpool.tile([P, KT, N], F16, name=f"ee_{b}")
                nc.scalar.dma_start(t, eet[:, b])
                ee01.append(t)
            xsb = cpool.tile([P, KT, BPC], F16)
            nc.scalar.dma_start(xsb, xt)
            bsb = cpool.tile([1, D], F16)
            nc.scalar.dma_start(bsb, bp)
            sel = cpool.tile([BPC, BPC * P], F16)
            nc.scalar.dma_start(sel, seld)
            wpsb = cpool.tile([P, KT, C], F16)
            nc.scalar.dma_start(wpsb, wp)
            ee23 = cpool.tile([P, 2, KT, N], F16)
            nc.scalar.dma_start(ee23, eet[:, 2:4])

            def ee_at(b, k):
                return ee01[b][:, k] if b < 2 else ee23[:, b - 2, k]

            # ---- constants ----
            ident_f = cpool.tile([P, P], F32)
            make_identity(nc, ident_f)
            ident = cpool.tile([P, P], F16)
            nc.scalar.copy(ident, ident_f)
            ones_f = cpool.tile([1, P], F32)
            nc.gpsimd.memset(ones_f, 1.0)
            ones = cpool.tile([1, P], F16)
            nc.scalar.copy(ones, ones_f)

            zsb = cpool.tile([BPC, C], F16)
            uT = cpool.tile([P, KT * BPC], F16)
            ysb = cpool.tile([BPC, C], F16)
            ybc = cpool.tile([P, NEARLY // NT, C], F32)

            o32 = []

            def utter_zu():
                # z = x @ W_proj + b
                for h in range(2):
                    cs = slice(h * FD, (h + 1) * FD)
                    zp = mm_ps.tile([P, FD], F32, tag="mm", name=f"z_{h}")
                    for k in range(KT):
                        nc.tensor.matmul(
                            zp[:BPC], xsb[:, k, :], wpsb[:, k, cs],
                            start=(k == 0), stop=False,
                        )
                    nc.tensor.matmul(
                        zp[:BPC], ones[:1, :BPC], bsb[:1, cs],
                        start=False, stop=True,
                    )
                    nc.vector.tensor_copy(zsb[:, cs], zp[:BPC])
                # transpose z into one psum tile, single Gelu -> uT
                tp = tp_ps.tile([P, KT * BPC], F16, tag="tp")
                for k in range(KT):
                    nc.tensor.transpose(
                        tp[:, k * BPC : (k + 1) * BPC],
                        zsb[:BPC, k * P : (k + 1) * P],
                        ident[:BPC, :BPC],
                    )
                nc.scalar.activation(uT, tp, GELU)

            def utter_y():
                # y = u @ W_u -> fp16 in partitions 0..3
                for h in range(2):
                    cs = slice(h * FD, (h + 1) * FD)
                    yp = mm_ps.tile([P, FD], F32, tag="mm", name=f"y_{h}")
                    for k in range(KT):
                        nc.tensor.matmul(
                            yp[:BPC], uT[:, k * BPC : (k + 1) * BPC],
                            wusb[:, k, cs],
                            start=(k == 0), stop=(k == KT - 1),
                        )
                    nc.vector.tensor_copy(ysb[:, cs], yp[:BPC])
                # ybc[:, b, :] = y[b] broadcast, for the early tiles' epilogue
                for b2 in range(NEARLY // NT):
                    for ch in range(2):
                        cs = slice(ch * FD, (ch + 1) * FD)
                        bp_ = mm_ps.tile([P, FD], F32, tag="mm", name=f"yb{b2}{ch}")
                        nc.tensor.matmul(
                            bp_, sel[:, b2 * P : (b2 + 1) * P], ysb[:BPC, cs],
                            start=True, stop=True,
                        )
                        if ch == 0:
                            nc.scalar.copy(ybc[:, b2, cs], bp_)
                        else:
                            nc.vector.tensor_copy(ybc[:, b2, cs], bp_)

            def store(mt, o):
                b, nh = divmod(mt, NT)
                ns = slice(nh * P, (nh + 1) * P)
                eng = nc.sync if mt % 2 == 0 else nc.scalar
                eng.dma_start(out[b, ns, :], o.rearrange("p a f -> p (a f)"))

            for mt in range(MT):
                if mt == 3:
                    utter_zu()
                if mt == 4:
                    utter_y()
                    # hoisted epilogue: add y to the early tiles on the DVE
                    # and store them, while the PE works on mt4..7
                    for emt in range(NEARLY):
                        eb = emt // NT
                        o = ostpool.tile([P, 2, FD], F16, tag="ost", name=f"oste{emt}")
                        nc.vector.tensor_add(
                            o[:, 0, :], o32[emt][:, 0, :], ybc[:, eb, 0:FD]
                        )
                        nc.vector.tensor_add(
                            o[:, 1, :], o32[emt][:, 1, :], ybc[:, eb, FD:C]
                        )
                        store(emt, o)

                b, nh = divmod(mt, NT)
                ns = slice(nh * P, (nh + 1) * P)
                fuse_y = mt >= NEARLY
                mps = [
                    mm_ps.tile([P, FD], F32, tag="mm", name=f"mm_{mt}_{ch}")
                    for ch in range(2)
                ]
                for ch in range(2):
                    cs = slice(ch * FD, (ch + 1) * FD)
                    for k in range(KT):
                        nc.tensor.matmul(
                            mps[ch],
                            ee_at(b, k)[:, ns],
                            we_at(k)[:, cs],
                            start=(k == 0),
                            stop=(False if fuse_y else k == KT - 1),
                        )
                    if fuse_y:
                        # fuse the y broadcast-add into the accumulation
                        nc.tensor.matmul(
                            mps[ch], sel[:, b * P : (b + 1) * P], ysb[:BPC, cs],
                            start=False, stop=True,
                        )
                if mt < NEARLY:
                    # y not ready: drain to f32, add y in the hoisted epilogue
                    o = outpool.tile([P, 2, FD], F32, tag=f"o{mt}")
                    nc.scalar.copy(o[:, 0, :], mps[0])
                    nc.vector.tensor_copy(o[:, 1, :], mps[1])
                    o32.append(o)
                else:
                    # PSUM holds the final logits: drain fp16 on both engines
                    o = ostpool.tile([P, 2, FD], F16, tag="ost", name=f"ost{mt}")
                    nc.scalar.copy(o[:, 0, :], mps[0])
                    nc.vector.tensor_copy(o[:, 1, :], mps[1])
                    store(mt, o)

    nc.compile()
    _CACHE["nc"] = nc
    return nc


def _prep(inputs):
    """Host-side cast to fp16 + k-chunk layout transforms."""
    x = np.asarray(inputs["encoded_utterance"], np.float32)
    ee = np.asarray(inputs["element_embeddings"], np.float32)
    w = np.asarray(inputs["weight_matrix"], np.float32)
    wp = np.asarray(inputs["W_proj"], np.float32)
    bp = np.asarray(inputs["b_proj"], np.float32).reshape(1, D)

    # eet[p, b, k, n] = ee[b, n, k*128+p]  (partition-major)
    eet = np.ascontiguousarray(
        ee.reshape(B, N, KT, P).transpose(3, 0, 2, 1)
    ).astype(np.float16)

    # we/wu/wp[p, k, c] = W[k*128+p, c]
    def kchunk(m):
        return np.ascontiguousarray(
            m.reshape(KT, P, C).transpose(1, 0, 2)
        ).astype(np.float16)

    we_h = kchunk(w[D:])
    wu_h = kchunk(w[:D])
    wp_h = kchunk(wp)
    bp_h = bp.astype(np.float16)
    # xt[p, k, b] = x[b, k*128+p], per-core slice of b
    xt_full = np.ascontiguousarray(
        x.reshape(B, KT, P).transpose(2, 1, 0)
    ).astype(np.float16)
    sel_h = np.kron(np.eye(BPC), np.ones((1, P))).astype(np.float16)
    return eet, we_h, wu_h, wp_h, bp_h, xt_full, sel_h


def run(inputs, trace=False, **kwargs):
    nc = _build()
    eet, we_h, wu_h, wp_h, bp_h, xt_full, sel_h = _prep(inputs)

    in_maps = []
    for i in range(NCORES):
        bs = slice(i * BPC, (i + 1) * BPC)
        in_maps.append(
            {
                "eet": np.ascontiguousarray(eet[:, bs]),
                "we": we_h,
                "wu": wu_h,
                "wp": wp_h,
                "xt": np.ascontiguousarray(xt_full[:, :, bs]),
                "bp": bp_h,
                "sel": sel_h,
            }
        )

    res = run_bass_kernel_spmd(
        nc, in_maps, core_ids=list(range(NCORES)), trace=trace, **kwargs
    )
    full = np.concatenate([r["logits"] for r in res.results], axis=0)
    return full.astype(np.float32), res


def kernel(**inputs) -> np.ndarray:
    return run(inputs, trace=False)[0]
